# revision 1
# baseline (speedup 1.0000x reference)
"""Trainium2 Bass kernel for the non-local-block module (nn_CNL_747324309589).

Sharding: data-parallel over batch — 16 batches across 8 NeuronCores, 2 per
core, no collectives.  Per batch (dims: HIGH=2048, LOW=512, N=H*W=1152):

    theta_xT[n,c] = sum_h xh[h,n]·thwT[h,c] + thb[c]      (x_h chunks = lhsT)
    phi_xT [n,d]  = sum_l xl[l,n]·phwT[l,d] + phb[d]      (phw,phb pre-scaled by 1/512)
    g_x    [d,n]  = sum_l gwT[l,d]·xl[l,n]  + gb[d]
    attT   [d,c]  = sum_n phi_xT[n,d]·theta_xT[n,c]       (= energy^T/512)
    y      [c,n]  = sum_d attT[d,c]·g_x[d,n]
    w_y    [o,n]  = sum_c wwT[c,o]·y[c,n]                 (BN scale pre-folded into ww)
    out    [o,n]  = w_y + bnt[o] + xh[o,n]                (shift + residual in one DVE op)

All matmuls run as float32r (full-rate PE at moving-dim >= 256) accumulating
fp32 in PSUM.  x_h stays resident in SBUF per batch (16x[128,1152]) serving
both as theta's lhsT chunks and the residual.  theta_wT and w_wT share one
4MB SBUF slot (disjoint phases, quarter-aligned byte ranges for progressive
turnover) so everything fits in the 28MiB SBUF.
"""

import numpy as np

import concourse.bass as bass
import concourse.bacc as bacc
import concourse.mybir as mybir
import concourse.tile as tile
from concourse.bass import ts

B, HIGH, LOW, H, W = 16, 2048, 512, 48, 24
N = H * W            # 1152
NCORES = 8
BPC = B // NCORES    # 2 batches per core
P = 128
KH = HIGH // P       # 16
KL = LOW // P        # 4
MN = N // P          # 9
NSPLIT = 3
NW = N // NSPLIT     # 384 (>=256 keeps float32r at full rate)
BN_EPS = 1e-5

F32 = mybir.dt.float32
F32R = mybir.dt.float32r
BF16 = mybir.dt.bfloat16
ADD = mybir.AluOpType.add
MULT = mybir.AluOpType.mult
AF = mybir.ActivationFunctionType


def _r(ap):
    return ap.bitcast(F32R)


def _build_module() -> bass.Bass:
    nc = bacc.Bacc()
    x_h = nc.dram_tensor("x_h", [BPC, HIGH, N], F32R, kind="ExternalInput")
    x_l = nc.dram_tensor("x_l", [BPC, LOW, N], F32R, kind="ExternalInput")
    thw = nc.dram_tensor("thw", [P, KH, LOW], F32R, kind="ExternalInput")
    phw = nc.dram_tensor("phw", [P, KL, LOW], F32R, kind="ExternalInput")
    gw = nc.dram_tensor("gw", [P, KL, LOW], F32R, kind="ExternalInput")
    # ww laid out [P, o-quarter, KL, 512] so each quarter is byte-aligned with
    # a thw k-quarter in the shared SBUF slot (progressive slot turnover)
    ww = nc.dram_tensor("ww", [P, 4, KL, HIGH // 4], F32R, kind="ExternalInput")
    thpb = nc.dram_tensor("thpb", [1, 2 * LOW], BF16, kind="ExternalInput")
    gbnt = nc.dram_tensor("gbnt", [P, KL + KH], F32, kind="ExternalInput")
    out = nc.dram_tensor("out", [BPC, HIGH, N], F32, kind="ExternalOutput")

    with tile.TileContext(nc) as tc:
        with (
            tc.tile_pool(name="consts", bufs=1) as cpool,
            tc.tile_pool(name="bigw", bufs=1) as wpool,
            tc.tile_pool(name="xh", bufs=KH) as xhpool,
            tc.tile_pool(name="xl", bufs=1) as xlpool,
            tc.tile_pool(name="mid", bufs=1) as midpool,
            tc.tile_pool(name="stg", bufs=9) as stgpool,
            tc.tile_pool(name="psum", bufs=8, space="PSUM") as pspool,
        ):
            # first batch's x_l and the phi weights go first so phase A2 can
            # start as early as possible; constants are packed into few DMAs
            # because serialized DMA-issue time paces the prologue
            xl0_sb = xlpool.tile([P, KL, N], F32R, tag="xl")
            nc.sync.dma_start(xl0_sb[:], x_l[0].rearrange("(ko p) n -> p ko n", p=P))
            phw_sb = cpool.tile([P, KL, LOW], F32R, tag="phw")
            nc.sync.dma_start(phw_sb[:], phw[:])
            gw_sb = cpool.tile([P, KL, LOW], F32R, tag="gw")
            nc.sync.dma_start(gw_sb[:], gw[:])
            thpb_sb = cpool.tile([P, 2 * LOW], BF16, tag="thpb")
            nc.sync.dma_start(thpb_sb[:], thpb[:].to_broadcast((P, 2 * LOW)))
            thb_sb = thpb_sb[:, :LOW]
            phb_sb = thpb_sb[:, LOW:]
            gbnt_sb = cpool.tile([P, KL + KH], F32, tag="gbnt")
            nc.sync.dma_start(gbnt_sb[:], gbnt[:])
            gb_sb = gbnt_sb[:, :KL]
            bnt_sb = gbnt_sb[:, KL:]

            for b in range(BPC):
                if b == 0:
                    xl_sb = xl0_sb
                else:
                    xl_sb = xlpool.tile([P, KL, N], F32R, tag="xl")
                    nc.sync.dma_start(
                        xl_sb[:], x_l[b].rearrange("(ko p) n -> p ko n", p=P)
                    )
                # interleave theta-weight quarters with x_h chunks so the
                # theta k-loop can start as soon as the first pieces land
                thw_sb = wpool.tile([P, KH, LOW], F32R, tag="bigw")
                xh_t = []
                for q in range(4):
                    nc.sync.dma_start(
                        thw_sb[:, ts(q, KH // 4), :], thw[:, ts(q, KH // 4), :]
                    )
                    for k in range(q * 4, q * 4 + 4):
                        t_ = xhpool.tile([P, N], F32R, tag="xh")
                        nc.sync.dma_start(t_[:], x_h[b, ts(k, P), :])
                        xh_t.append(t_)

                # phi_xT [n, d] (phase A2)
                ph_sb = midpool.tile([P, MN, LOW], F32R, tag="ph")
                for m in range(MN):
                    ps = pspool.tile([P, 512], F32, tag="ps")
                    for k in range(KL):
                        nc.tensor.matmul(
                            ps[:],
                            _r(xl_sb[:, k, ts(m, P)]),
                            _r(phw_sb[:, k, :]),
                            start=(k == 0),
                            stop=(k == KL - 1),
                        )
                    nc.vector.tensor_tensor(ph_sb[:, m, :], ps[:], phb_sb[:], ADD)

                # g_x [d, n] (phase A3)
                g_sb = midpool.tile([P, KL, N], F32R, tag="g")
                for md in range(KL):
                    for nn in range(NSPLIT):
                        ps = pspool.tile([P, 512], F32, tag="ps")
                        for k in range(KL):
                            nc.tensor.matmul(
                                ps[:, :NW],
                                _r(gw_sb[:, k, ts(md, P)]),
                                _r(xl_sb[:, k, ts(nn, NW)]),
                                start=(k == 0),
                                stop=(k == KL - 1),
                            )
                        nc.scalar.activation(
                            g_sb[:, md, ts(nn, NW)],
                            ps[:, :NW],
                            AF.Identity,
                            bias=gb_sb[:, md : md + 1],
                        )

                # theta_xT [n, c] (phase A1)
                th_sb = midpool.tile([P, MN, LOW], F32R, tag="th")
                for m in range(MN):
                    ps = pspool.tile([P, 512], F32, tag="ps")
                    for k in range(KH):
                        nc.tensor.matmul(
                            ps[:],
                            _r(xh_t[k][:, ts(m, P)]),
                            _r(thw_sb[:, k, :]),
                            start=(k == 0),
                            stop=(k == KH - 1),
                        )
                    nc.vector.tensor_tensor(th_sb[:, m, :], ps[:], thb_sb[:], ADD)

                # attT [d, c] = energy^T/512 (phase B1); parks in the xl slot
                # (xl is dead after A3, reloaded for b+1 only after B2 reads)
                att_sb = xlpool.tile([P, KL, LOW], F32R, tag="xl")
                for md in range(KL):
                    ps = pspool.tile([P, 512], F32, tag="ps")
                    for k in range(MN):
                        nc.tensor.matmul(
                            ps[:],
                            _r(ph_sb[:, k, ts(md, P)]),
                            _r(th_sb[:, k, :]),
                            start=(k == 0),
                            stop=(k == MN - 1),
                        )
                    nc.scalar.activation(att_sb[:, md, :], ps[:], AF.Copy)

                # y [c, n] (phase B2)
                # y shares the theta_xT slot: th is dead after B1, same byte size
                y_sb = midpool.tile([P, KL, N], F32R, tag="th")
                for mc in range(KL):
                    for nn in range(NSPLIT):
                        ps = pspool.tile([P, 512], F32, tag="ps")
                        for k in range(KL):
                            nc.tensor.matmul(
                                ps[:, :NW],
                                _r(att_sb[:, k, ts(mc, P)]),
                                _r(g_sb[:, k, ts(nn, NW)]),
                                start=(k == 0),
                                stop=(k == KL - 1),
                            )
                        nc.scalar.activation(y_sb[:, mc, ts(nn, NW)], ps[:, :NW], AF.Copy)

                # w_y + BN + residual (phase C)
                ww_sb = wpool.tile([P, 4, KL, HIGH // 4], F32R, tag="bigw")
                for q in range(4):
                    nc.sync.dma_start(ww_sb[:, q], ww[:, q])
                for mo in range(KH):
                    xt = xh_t[mo]
                    for nn in range(NSPLIT):
                        ps = pspool.tile([P, 512], F32, tag="ps")
                        for k in range(KL):
                            nc.tensor.matmul(
                                ps[:, :NW],
                                _r(ww_sb[:, mo // 4, k, ts(mo % 4, P)]),
                                _r(y_sb[:, k, ts(nn, NW)]),
                                start=(k == 0),
                                stop=(k == KL - 1),
                            )
                        stg = stgpool.tile([P, NW], F32, tag="stg")
                        nc.vector.scalar_tensor_tensor(
                            stg[:],
                            ps[:, :NW],
                            bnt_sb[:, mo : mo + 1],
                            xt[:, ts(nn, NW)].bitcast(F32),
                            ADD,
                            ADD,
                        )
                        nc.sync.dma_start(out[b, ts(mo, P), ts(nn, NW)], stg[:])
    nc.compile()
    return nc


_CACHE: dict = {}


def _get_module() -> bass.Bass:
    if "nc" not in _CACHE:
        _CACHE["nc"] = _build_module()
    return _CACHE["nc"]


def _prep_maps(inputs: dict) -> list[dict]:
    f = lambda a: np.ascontiguousarray(np.asarray(a, dtype=np.float32))
    x_h = f(inputs["x_h"]).reshape(B, HIGH, N)
    x_l = f(inputs["x_l"]).reshape(B, LOW, N)
    theta_w = f(inputs["theta_w"])
    phi_w = f(inputs["phi_w"])
    g_w = f(inputs["g_w"])
    w_w = f(inputs["w_w"])

    thw_h = np.ascontiguousarray(theta_w.T.reshape(KH, P, LOW).transpose(1, 0, 2))
    phw_h = np.ascontiguousarray((phi_w.T / np.float32(LOW)).reshape(KL, P, LOW).transpose(1, 0, 2))
    gw_h = np.ascontiguousarray(g_w.T.reshape(KL, P, LOW).transpose(1, 0, 2))
    s = f(inputs["bn_gamma"]) / np.sqrt(f(inputs["bn_var"]) + np.float32(BN_EPS))
    # BN scale folded into the w conv weights; only the shift remains on-device
    ww_h = np.ascontiguousarray(
        (w_w * s[:, None])
        .astype(np.float32)
        .T.reshape(KL, P, 4, HIGH // 4)
        .transpose(1, 2, 0, 3)
    )

    import ml_dtypes
    thpb_h = np.concatenate(
        [f(inputs["theta_b"]), f(inputs["phi_b"]) / np.float32(LOW)]
    ).reshape(1, 2 * LOW).astype(ml_dtypes.bfloat16)
    gb_h = np.ascontiguousarray(f(inputs["g_b"]).reshape(KL, P).T)
    t = (f(inputs["w_b"]) - f(inputs["bn_mean"])) * s + f(inputs["bn_beta"])
    bnt_h = np.ascontiguousarray(t.astype(np.float32).reshape(KH, P).T)
    gbnt_h = np.ascontiguousarray(np.concatenate([gb_h, bnt_h], axis=1))

    shared = dict(
        thw=thw_h, phw=phw_h, gw=gw_h, ww=ww_h, thpb=thpb_h, gbnt=gbnt_h,
    )
    maps = []
    for c in range(NCORES):
        m = dict(shared)
        m["x_h"] = np.ascontiguousarray(x_h[c * BPC : (c + 1) * BPC])
        m["x_l"] = np.ascontiguousarray(x_l[c * BPC : (c + 1) * BPC])
        maps.append(m)
    return maps


def _run(inputs: dict, **kwargs):
    from concourse.bass_utils import run_bass_kernel_spmd

    nc = _get_module()
    in_maps = _prep_maps(inputs)
    res = run_bass_kernel_spmd(nc, in_maps, core_ids=list(range(NCORES)), **kwargs)
    parts = [r["out"] for r in res.results]
    full = np.concatenate(parts, axis=0).reshape(B, HIGH, H, W)
    return full, res


def kernel(**inputs) -> np.ndarray:
    full, _ = _run(inputs)
    return full



# revision 32
# speedup vs baseline: 1.3798x; 1.3798x over previous
"""Trainium2 Bass kernel for the non-local-block module (nn_CNL_747324309589).

Sharding: data-parallel over batch — 16 batches across 8 NeuronCores, 2 per
core, no collectives.  Per batch (dims: HIGH=2048, LOW=512, N=H*W=1152):

    theta_xT[n,c] = sum_h xh[h,n]·thwT[h,c] + thb[c]
    phi_xT [n,d]  = sum_l xl[l,n]·phwT[l,d] + phb[d]     (evict folds /512)
    g_x    [d,n]  = sum_l gwT[l,d]·xl[l,n]  + gb[d]
    attT   [d,c]  = sum_n phi_xT[n,d]·theta_xT[n,c]      (= energy^T/512)
    y      [c,n]  = sum_d attT[d,c]·g_x[d,n]
    w_y    [o,n]  = sum_c wwT[c,o]·y[c,n]                (BN scale in ww)
    out    [o,n]  = w_y + (xh[o,n] + bnt[o])             (bnt folded into the
                                                          bf16 residual copy)

ALL six matmuls run as fp8e4 DoubleRow pairs at 0.5 PE-cycles per moving
row.  Each operand is split hi+lo in fp8 (double-fp8 ~11 mantissa bits) and
each product keeps 3 of the 4 cross terms:

    W^T X ~= W_hi^T(X_hi + X_lo) + W_lo^T X_hi            (lo·lo dropped)

One DoubleRow instr computes A^T i0 + B^T i1, so per 128-deep k-chunk the
paired-plane side does one instr per k and the lo-correction parts of two
adjacent k-chunks share one instr via strided APs.  Net 0.75 cycles per
128-contraction row vs 1.0 for fp32r, at ~0.4% rel error (gate is 2e-2).
Everything is pre-scaled by powers of two (weights x16, x x4, th x8,
ph x4096, g x16, att x64, y x2) so fp8 lo planes stay in e4m3 normal range
and every eviction absorbs the inverse scale in its scalar multiplier.
Device-side hi/lo splits (th/ph/g/att/y) cost one extra Act cast + one Pool
subtract per tile, spread so no engine paces the PE.  x_h arrives as
host-split fp8 planes plus a bf16 residual copy with the BN shift
pre-added; output is written bf16 and upcast on host.

Schedule notes: the PE p-state ramp is burned on dummy matmuls during the
prologue DMA wait; batch-0 theta inputs (thw, xh hi plane, then xh lo
plane) stream before phase-C inputs (ww, xhb), and theta's lo-correction
instrs are issued first since they only need the hi plane.
"""

import numpy as np

import concourse.bass as bass
import concourse.bacc as bacc
import concourse.mybir as mybir
import concourse.tile as tile
from concourse.bass import ts

B, HIGH, LOW, H, W = 16, 2048, 512, 48, 24
N = H * W            # 1152
NCORES = 8
BPC = B // NCORES    # 2 batches per core
P = 128
KH = HIGH // P       # 16
KL = LOW // P        # 4
MN = N // P          # 9
NSPLIT = 3
NW = N // NSPLIT     # 384
BN_EPS = 1e-5

SW = 16.0            # weight fp8 pre-scale
SX = 4.0             # x_h / x_l fp8 pre-scale
STH = 8.0            # theta_xT fp8 split scale
SPH = 4096.0         # phi_xT fp8 split scale (on top of the /512 fold)
SG = 16.0            # g_x fp8 split scale
SATT = 64.0          # att fp8 split scale
SY = 2.0             # y fp8 split scale

F32 = mybir.dt.float32
BF16 = mybir.dt.bfloat16
FP8 = mybir.dt.float8e4
DR = mybir.MatmulPerfMode.DoubleRow
ADD = mybir.AluOpType.add
MULT = mybir.AluOpType.mult
SUB = mybir.AluOpType.subtract
AF = mybir.ActivationFunctionType


def _dup(ap, n):
    # (W, W) stride-0 pair for one slot of a DoubleRow operand
    return ap.unsqueeze(1).to_broadcast((P, 2, n))


def _build_module() -> bass.Bass:
    nc = bacc.Bacc()
    xhf8 = nc.dram_tensor("xhf8", [BPC, P, 2, KH, N], FP8, kind="ExternalInput")
    xhb = nc.dram_tensor("xhb", [BPC, P, KH, N], BF16, kind="ExternalInput")
    xlf8 = nc.dram_tensor("xlf8", [BPC, P, 2, KL, N], FP8, kind="ExternalInput")
    thwh = nc.dram_tensor("thwh", [P, KH, LOW], FP8, kind="ExternalInput")
    thwl = nc.dram_tensor("thwl", [P, KH, LOW], FP8, kind="ExternalInput")
    phwh = nc.dram_tensor("phwh", [P, KL, LOW], FP8, kind="ExternalInput")
    phwl = nc.dram_tensor("phwl", [P, KL, LOW], FP8, kind="ExternalInput")
    gwh = nc.dram_tensor("gwh", [P, KL, LOW], FP8, kind="ExternalInput")
    gwl = nc.dram_tensor("gwl", [P, KL, LOW], FP8, kind="ExternalInput")
    wwh = nc.dram_tensor("wwh", [P, KL, HIGH], FP8, kind="ExternalInput")
    wwl = nc.dram_tensor("wwl", [P, KL, HIGH], FP8, kind="ExternalInput")
    thpb = nc.dram_tensor("thpb", [1, 2 * LOW], BF16, kind="ExternalInput")
    gb = nc.dram_tensor("gb", [P, KL], F32, kind="ExternalInput")
    bnt = nc.dram_tensor("bnt", [P, KH], F32, kind="ExternalInput")
    ident = nc.dram_tensor("ident", [P, 2, P], FP8, kind="ExternalInput")
    out = nc.dram_tensor("out", [BPC, HIGH, N], BF16, kind="ExternalOutput")

    s_a1 = STH / (SW * SX)             # A1 evict: psum -> 8*theta_xT   (1/8)
    s_a2 = SPH / (SW * SX * LOW)       # A2 evict: psum -> 4096*ph      (1/8)
    s_a3 = SG / (SW * SX)              # A3 evict: psum -> 16*g_x       (1/4)
    s_b1 = SATT / (STH * SPH)          # B1 evict: psum -> 64*att       (2^-9)
    s_b2 = SY / (SATT * SG)            # B2 evict: psum -> 2*y          (2^-9)
    inv_c = 1.0 / (SW * SY)            # C evict                        (1/32)

    with tile.TileContext(nc) as tc:
        with (
            tc.tile_pool(name="consts", bufs=1) as cpool,
            tc.tile_pool(name="xh8", bufs=2) as xh8pool,
            tc.tile_pool(name="xhb", bufs=4) as xhbpool,
            tc.tile_pool(name="xl", bufs=1) as xlpool,
            tc.tile_pool(name="mid", bufs=1) as midpool,
            tc.tile_pool(name="tmp", bufs=3) as tmppool,
            tc.tile_pool(name="stg", bufs=4) as stgpool,
            tc.tile_pool(name="psum", bufs=8, space="PSUM") as pspool,
        ):
            # prologue: biases first (the A2 evictions need them right after
            # the first matmuls), then phw, then xl in two n-halves: A2's
            # m-groups read only their own n-columns, so the first half
            # unblocks m=0..3
            thpb_sb = cpool.tile([P, 2 * LOW], BF16, tag="thpb")
            nc.sync.dma_start(thpb_sb[:], thpb[:].to_broadcast((P, 2 * LOW)))
            thb_sb = thpb_sb[:, :LOW]
            phb_sb = thpb_sb[:, LOW:]
            gb_sb = cpool.tile([P, KL], F32, tag="gb")
            nc.sync.dma_start(gb_sb[:], gb[:])
            phwh_sb = cpool.tile([P, KL, LOW], FP8, tag="phwh")
            nc.sync.dma_start(phwh_sb[:], phwh[:])
            phwl_sb = cpool.tile([P, KL, LOW], FP8, tag="phwl")
            nc.sync.dma_start(phwl_sb[:], phwl[:])
            xl0_sb = xlpool.tile([P, 2, KL, N], FP8, tag="xl")
            for h in range(2):
                nc.sync.dma_start(
                    xl0_sb[:, :, :, ts(h, N // 2)], xlf8[0, :, :, :, ts(h, N // 2)]
                )
            gwh_sb = cpool.tile([P, KL, LOW], FP8, tag="gwh")
            nc.sync.dma_start(gwh_sb[:], gwh[:])
            gwl_sb = cpool.tile([P, KL, LOW], FP8, tag="gwl")
            nc.sync.dma_start(gwl_sb[:], gwl[:])
            bnt_sb = cpool.tile([P, KH], F32, tag="bnt")
            nc.sync.dma_start(bnt_sb[:], bnt[:])
            ident_sb = cpool.tile([P, 2, P], FP8, tag="ident")
            nc.sync.dma_start(ident_sb[:], ident[:])
            # theta weights interleaved with batch-0 xh fp8 planes: these gate
            # phase A1 of batch 0, so they go before ww/xhb (phase-C inputs).
            # hi plane streams fully before lo: A1's lo-correction instrs only
            # need the hi plane, so issuing them first (below) lets the PE
            # keep pace with DMA arrival
            thwh_sb = cpool.tile([P, KH, LOW], FP8, tag="thwh")
            thwl_sb = cpool.tile([P, KH, LOW], FP8, tag="thwl")
            xh8_b0 = xh8pool.tile([P, 2, KH, N], FP8, tag="xh8")
            for q in range(4):
                nc.sync.dma_start(
                    thwh_sb[:, ts(q, KH // 4)], thwh[:, ts(q, KH // 4)]
                )
                nc.sync.dma_start(
                    thwl_sb[:, ts(q, KH // 4)], thwl[:, ts(q, KH // 4)]
                )
                nc.sync.dma_start(
                    xh8_b0[:, 0, ts(q, KH // 4)], xhf8[0, :, 0, ts(q, KH // 4)]
                )
            for q in range(4):
                nc.sync.dma_start(
                    xh8_b0[:, 1, ts(q, KH // 4)], xhf8[0, :, 1, ts(q, KH // 4)]
                )
            wwh_sb = cpool.tile([P, KL, HIGH], FP8, tag="wwh")
            nc.sync.dma_start(wwh_sb[:], wwh[:])
            wwl_sb = cpool.tile([P, KL, HIGH], FP8, tag="wwl")
            nc.sync.dma_start(wwl_sb[:], wwl[:])

            # warm the PE p-state during the prologue DMA wait: ~4.7us of dummy
            # matmuls on a memset tile burn the half-clock ramp window so real
            # work starts at full clock (sized to end just before xl lands)
            warm = cpool.tile([P, 640], FP8, tag="warm")
            nc.vector.memset(warm[:], 0.0)
            wps = pspool.tile([P, 512], F32, tag="ps")
            warm_l = warm[:, :256].rearrange("p (two m) -> p two m", two=2)
            warm_r = warm[:, :512].unsqueeze(1).to_broadcast((P, 2, 512))
            for i in range(34):
                nc.tensor.matmul(
                    wps[:],
                    warm_l,
                    warm_r,
                    start=(i == 0),
                    stop=(i == 33),
                    perf_mode=DR,
                )

            for b in range(BPC):
                if b == 0:
                    xl_sb = xl0_sb
                    xh8_sb = xh8_b0
                else:
                    xl_sb = xlpool.tile([P, 2, KL, N], FP8, tag="xl")
                    nc.sync.dma_start(xl_sb[:], xlf8[b])
                    xh8_sb = xh8pool.tile([P, 2, KH, N], FP8, tag="xh8")
                    for pl in range(2):
                        for q in range(4):
                            nc.sync.dma_start(
                                xh8_sb[:, pl, ts(q, KH // 4)],
                                xhf8[b, :, pl, ts(q, KH // 4)],
                            )
                xhb_t = []
                for q in range(4):
                    t_ = xhbpool.tile([P, KH // 4, N], BF16, tag="xhb")
                    nc.sync.dma_start(t_[:], xhb[b, :, ts(q, KH // 4)])
                    xhb_t.append(t_)

                # phi_xT planes [n, d] (A2): stationary xl pair, moving phw.
                # ph8 has a zeroed 10th k-slot so B1's odd lo-instr can pair
                # (ph_hi[8], 0)
                ph8 = midpool.tile([P, 2, MN + 1, LOW], FP8, tag="ph")
                nc.vector.memset(ph8[:, 0, MN, :], 0.0)
                for m in range(MN):
                    ps = pspool.tile([P, 512], F32, tag="ps")
                    for k in range(KL):
                        nc.tensor.matmul(
                            ps[:],
                            xl_sb[:, :, k, ts(m, P)],
                            _dup(phwh_sb[:, k, :], LOW),
                            start=(k == 0),
                            stop=False,
                            perf_mode=DR,
                        )
                    for j in range(KL // 2):
                        nc.tensor.matmul(
                            ps[:],
                            xl_sb[:, 0, ts(j, 2), ts(m, P)],
                            phwl_sb[:, ts(j, 2)],
                            start=False,
                            stop=(j == KL // 2 - 1),
                            perf_mode=DR,
                        )
                    tmp = tmppool.tile([P, 512], F32, tag="tmp")
                    nc.vector.scalar_tensor_tensor(
                        tmp[:], ps[:], s_a2, phb_sb, MULT, ADD
                    )
                    nc.scalar.activation(ph8[:, 0, m, :], tmp[:], AF.Copy)
                    nc.gpsimd.scalar_tensor_tensor(
                        ph8[:, 1, m, :], tmp[:], 1.0, ph8[:, 0, m, :], MULT, SUB
                    )

                # g_x planes [d, n] (A3): stationary gw pair, moving xl planes
                g8 = midpool.tile([P, 2, KL, N], FP8, tag="g")
                for md in range(KL):
                    for nn in range(NSPLIT):
                        ps = pspool.tile([P, 512], F32, tag="ps")
                        for k in range(KL):
                            nc.tensor.matmul(
                                ps[:, :NW],
                                _dup(gwh_sb[:, k, ts(md, P)], P),
                                xl_sb[:, :, k, ts(nn, NW)],
                                start=(k == 0),
                                stop=False,
                                perf_mode=DR,
                            )
                        for j in range(KL // 2):
                            nc.tensor.matmul(
                                ps[:, :NW],
                                gwl_sb[:, ts(j, 2), ts(md, P)],
                                xl_sb[:, 0, ts(j, 2), ts(nn, NW)],
                                start=False,
                                stop=(j == KL // 2 - 1),
                                perf_mode=DR,
                            )
                        # tmp on DVE (broadcast bias) — two Act ops per tile
                        # would pace A3 at 1010ns/tile vs the PE's 480ns
                        tmp = tmppool.tile([P, 512], F32, tag="tmp")
                        nc.vector.scalar_tensor_tensor(
                            tmp[:, :NW],
                            ps[:, :NW],
                            s_a3,
                            gb_sb[:, md : md + 1].to_broadcast((P, NW)),
                            MULT,
                            ADD,
                        )
                        nc.scalar.activation(
                            g8[:, 0, md, ts(nn, NW)], tmp[:, :NW], AF.Copy
                        )
                        nc.gpsimd.scalar_tensor_tensor(
                            g8[:, 1, md, ts(nn, NW)],
                            tmp[:, :NW],
                            1.0,
                            g8[:, 0, md, ts(nn, NW)],
                            MULT,
                            SUB,
                        )

                # theta_xT planes [n, c] (A1): stationary xh planes, moving
                # thw, as three pair-across-k instr sets (hi·Whi, hi·Wlo,
                # lo·Whi).  The first two read only the hi plane, which lands
                # before the lo plane in the DMA stream, so the lo-dependent
                # set issues last and the PE keeps pace with DMA arrival
                th8 = midpool.tile([P, 2, MN, LOW], FP8, tag="th")
                for m in range(MN):
                    ps = pspool.tile([P, 512], F32, tag="ps")
                    for j in range(KH // 2):
                        nc.tensor.matmul(
                            ps[:],
                            xh8_sb[:, 0, ts(j, 2), ts(m, P)],
                            thwh_sb[:, ts(j, 2)],
                            start=(j == 0),
                            stop=False,
                            perf_mode=DR,
                        )
                    for j in range(KH // 2):
                        nc.tensor.matmul(
                            ps[:],
                            xh8_sb[:, 0, ts(j, 2), ts(m, P)],
                            thwl_sb[:, ts(j, 2)],
                            start=False,
                            stop=False,
                            perf_mode=DR,
                        )
                    for j in range(KH // 2):
                        nc.tensor.matmul(
                            ps[:],
                            xh8_sb[:, 1, ts(j, 2), ts(m, P)],
                            thwh_sb[:, ts(j, 2)],
                            start=False,
                            stop=(j == KH // 2 - 1),
                            perf_mode=DR,
                        )
                    tmp = tmppool.tile([P, 512], F32, tag="tmp")
                    nc.vector.scalar_tensor_tensor(
                        tmp[:], ps[:], s_a1, thb_sb, MULT, ADD
                    )
                    nc.scalar.activation(th8[:, 0, m, :], tmp[:], AF.Copy)
                    nc.gpsimd.scalar_tensor_tensor(
                        th8[:, 1, m, :], tmp[:], 1.0, th8[:, 0, m, :], MULT, SUB
                    )

                # attT planes [d, c] = energy^T/512 (B1): stationary ph pair,
                # moving th planes; att parks in the xl slot
                att8 = xlpool.tile([P, 2, KL, LOW], FP8, tag="xl")
                for md in range(KL):
                    ps = pspool.tile([P, 512], F32, tag="ps")
                    for k in range(MN):
                        nc.tensor.matmul(
                            ps[:],
                            ph8[:, :, k, ts(md, P)],
                            _dup(th8[:, 0, k, :], LOW),
                            start=(k == 0),
                            stop=False,
                            perf_mode=DR,
                        )
                    for j in range(MN // 2):
                        nc.tensor.matmul(
                            ps[:],
                            ph8[:, 0, ts(j, 2), ts(md, P)],
                            th8[:, 1, ts(j, 2), :],
                            start=False,
                            stop=False,
                            perf_mode=DR,
                        )
                    # odd 9th chunk: lhsT pairs (ph_hi[8], zero-slot-9)
                    nc.tensor.matmul(
                        ps[:],
                        ph8[:, 0, MN - 1 : MN + 1, ts(md, P)],
                        _dup(th8[:, 1, MN - 1, :], LOW),
                        start=False,
                        stop=True,
                        perf_mode=DR,
                    )
                    nc.scalar.activation(
                        att8[:, 0, md, :], ps[:], AF.Copy, scale=s_b1
                    )
                    nc.vector.scalar_tensor_tensor(
                        att8[:, 1, md, :], ps[:], s_b1, att8[:, 0, md, :], MULT, SUB
                    )

                # y planes [c, n] (B2): stationary att pair, moving g planes;
                # parks in the th slot
                y8 = midpool.tile([P, 2, KL, N], FP8, tag="th")
                for mc in range(KL):
                    for nn in range(NSPLIT):
                        ps = pspool.tile([P, 512], F32, tag="ps")
                        for k in range(KL):
                            nc.tensor.matmul(
                                ps[:, :NW],
                                _dup(att8[:, 0, k, ts(mc, P)], P),
                                g8[:, :, k, ts(nn, NW)],
                                start=(k == 0),
                                stop=False,
                                perf_mode=DR,
                            )
                        for j in range(KL // 2):
                            nc.tensor.matmul(
                                ps[:, :NW],
                                att8[:, 1, ts(j, 2), ts(mc, P)],
                                g8[:, 0, ts(j, 2), ts(nn, NW)],
                                start=False,
                                stop=(j == KL // 2 - 1),
                                perf_mode=DR,
                            )
                        nc.scalar.activation(
                            y8[:, 0, mc, ts(nn, NW)], ps[:, :NW], AF.Copy, scale=s_b2
                        )
                        nc.vector.scalar_tensor_tensor(
                            y8[:, 1, mc, ts(nn, NW)],
                            ps[:, :NW],
                            s_b2,
                            y8[:, 0, mc, ts(nn, NW)],
                            MULT,
                            SUB,
                        )

                # w_y + residual  (C): stationary ww pair, moving y planes.
                # stage a full [P, N] row per mo -> one out DMA per mo (the
                # SP sequencer spends ~0.5us per dma_start; 48 issues/batch
                # would pace the whole phase)
                for mo in range(KH):
                    xt = xhb_t[mo // 4]
                    stg = stgpool.tile([P, N], BF16, tag="stg")
                    for nn in range(NSPLIT):
                        ps = pspool.tile([P, 512], F32, tag="ps")
                        for k in range(KL):
                            nc.tensor.matmul(
                                ps[:, :NW],
                                _dup(wwh_sb[:, k, ts(mo, P)], P),
                                y8[:, :, k, ts(nn, NW)],
                                start=(k == 0),
                                stop=False,
                                perf_mode=DR,
                            )
                        for j in range(KL // 2):
                            nc.tensor.matmul(
                                ps[:, :NW],
                                wwl_sb[:, ts(j, 2), ts(mo, P)],
                                y8[:, 0, ts(j, 2), ts(nn, NW)],
                                start=False,
                                stop=(nn < 2 and j == KL // 2 - 1),
                                perf_mode=DR,
                            )
                        # DVE alone paces C (525ns/evict vs 480ns PE
                        # cadence) and GPSIMD cannot read PSUM, so the nn2
                        # third rides the PE: one DoubleRow instr with an
                        # 8I pair adds 32*xh (residual) into the psum, and
                        # Act evicts with the BN shift as its bias
                        if nn < 2:
                            nc.vector.scalar_tensor_tensor(
                                stg[:, ts(nn, NW)],
                                ps[:, :NW],
                                inv_c,
                                xt[:, mo % 4, ts(nn, NW)],
                                MULT,
                                ADD,
                            )
                        else:
                            nc.tensor.matmul(
                                ps[:, :NW],
                                ident_sb[:],
                                xh8_sb[:, :, mo, ts(nn, NW)],
                                start=False,
                                stop=True,
                                perf_mode=DR,
                            )
                            nc.scalar.activation(
                                stg[:, ts(nn, NW)],
                                ps[:, :NW],
                                AF.Identity,
                                bias=bnt_sb[:, mo : mo + 1],
                                scale=inv_c,
                            )
                    last = b == BPC - 1 and mo == KH - 1
                    if not last:
                        nc.sync.dma_start(out[b, ts(mo, P), :], stg[:])
                    else:
                        # final row: per-third DMAs so the first two fly while
                        # the last eviction drains
                        for nn in range(NSPLIT):
                            nc.sync.dma_start(
                                out[b, ts(mo, P), ts(nn, NW)], stg[:, ts(nn, NW)]
                            )
    nc.compile()
    return nc


_CACHE: dict = {}


def _get_module() -> bass.Bass:
    if "nc" not in _CACHE:
        _CACHE["nc"] = _build_module()
    return _CACHE["nc"]


def _split_fp8(x: np.ndarray, scale: float):
    import ml_dtypes

    E4 = ml_dtypes.float8_e4m3
    xs = (x * np.float32(scale)).astype(np.float32)
    hi = xs.astype(E4)
    lo = (xs - hi.astype(np.float32)).astype(E4)
    return hi, lo


def _prep_maps(inputs: dict) -> list[dict]:
    import ml_dtypes

    f = lambda a: np.ascontiguousarray(np.asarray(a, dtype=np.float32))
    x_h = f(inputs["x_h"]).reshape(B, HIGH, N)
    x_l = f(inputs["x_l"]).reshape(B, LOW, N)
    theta_w = f(inputs["theta_w"])
    phi_w = f(inputs["phi_w"])
    g_w = f(inputs["g_w"])
    w_w = f(inputs["w_w"])

    def wq(wmat, kk):
        # wmat [kk*P, F]; -> hi [P, kk, F] and lo [P, kk, F]
        hi, lo = _split_fp8(wmat, SW)
        hi = np.ascontiguousarray(hi.reshape(kk, P, -1).transpose(1, 0, 2))
        lo = np.ascontiguousarray(lo.reshape(kk, P, -1).transpose(1, 0, 2))
        return hi, lo

    thwh_h, thwl_h = wq(theta_w.T, KH)          # [HIGH, LOW] over h-chunks
    phwh_h, phwl_h = wq(phi_w.T, KL)            # [LOW, LOW]
    gwh_h, gwl_h = wq(g_w.T, KL)                # [LOW, LOW]
    s = f(inputs["bn_gamma"]) / np.sqrt(f(inputs["bn_var"]) + np.float32(BN_EPS))
    wwh_h, wwl_h = wq((w_w * s[:, None]).T, KL)  # [LOW, HIGH] over c-chunks

    def xq(x, kk):
        # x [Bn, kk*P, N] -> fp8 planes [Bn, P, 2, kk, N]
        hi, lo = _split_fp8(x, SX)
        hi = hi.reshape(-1, kk, P, N).transpose(0, 2, 1, 3)
        lo = lo.reshape(-1, kk, P, N).transpose(0, 2, 1, 3)
        return np.ascontiguousarray(np.stack([hi, lo], axis=2))

    xhf8_h = xq(x_h, KH)
    xlf8_h = xq(x_l, KL)

    t = (f(inputs["w_b"]) - f(inputs["bn_mean"])) * s + f(inputs["bn_beta"])
    xhb_h = np.ascontiguousarray(
        (x_h + t[None, :, None]).reshape(B, KH, P, N).transpose(0, 2, 1, 3)
    ).astype(ml_dtypes.bfloat16)
    bnt_h = np.ascontiguousarray(t.reshape(KH, P).T)
    ident_h = np.ascontiguousarray(
        np.broadcast_to((8.0 * np.eye(P, dtype=np.float32))[:, None, :], (P, 2, P))
    ).astype(ml_dtypes.float8_e4m3)

    # biases pre-scaled to the split scales: thb*STH, (phb/512)*SPH = 8*phb
    thpb_h = (
        np.concatenate(
            [
                f(inputs["theta_b"]) * np.float32(STH),
                f(inputs["phi_b"]) * np.float32(SPH / LOW),
            ]
        )
        .reshape(1, 2 * LOW)
        .astype(ml_dtypes.bfloat16)
    )
    gb_h = np.ascontiguousarray(
        (f(inputs["g_b"]) * np.float32(SG)).reshape(KL, P).T
    )

    shared = dict(
        thwh=thwh_h, thwl=thwl_h, phwh=phwh_h, phwl=phwl_h,
        gwh=gwh_h, gwl=gwl_h, wwh=wwh_h, wwl=wwl_h,
        thpb=thpb_h, gb=gb_h, bnt=bnt_h, ident=ident_h,
    )
    maps = []
    for c in range(NCORES):
        m = dict(shared)
        m["xhf8"] = np.ascontiguousarray(xhf8_h[c * BPC : (c + 1) * BPC])
        m["xlf8"] = np.ascontiguousarray(xlf8_h[c * BPC : (c + 1) * BPC])
        m["xhb"] = np.ascontiguousarray(xhb_h[c * BPC : (c + 1) * BPC])
        maps.append(m)
    return maps


def _run(inputs: dict, **kwargs):
    from concourse.bass_utils import run_bass_kernel_spmd

    nc = _get_module()
    in_maps = _prep_maps(inputs)
    res = run_bass_kernel_spmd(nc, in_maps, core_ids=list(range(NCORES)), **kwargs)
    parts = [np.asarray(r["out"], dtype=np.float32) for r in res.results]
    full = np.concatenate(parts, axis=0).reshape(B, HIGH, H, W)
    return full, res


def kernel(**inputs) -> np.ndarray:
    full, _ = _run(inputs)
    return full


# revision 39
# speedup vs baseline: 1.3920x; 1.0088x over previous
"""Trainium2 Bass kernel for the non-local-block module (nn_CNL_747324309589).

Sharding: data-parallel over batch — 16 batches across 8 NeuronCores, 2 per
core, no collectives.  Per batch (dims: HIGH=2048, LOW=512, N=H*W=1152):

    theta_xT[n,c] = sum_h xh[h,n]·thwT[h,c] + thb[c]
    phi_xT [n,d]  = sum_l xl[l,n]·phwT[l,d] + phb[d]     (evict folds /512)
    g_x    [d,n]  = sum_l gwT[l,d]·xl[l,n]  + gb[d]
    attT   [d,c]  = sum_n phi_xT[n,d]·theta_xT[n,c]      (= energy^T/512)
    y      [c,n]  = sum_d attT[d,c]·g_x[d,n]
    w_y    [o,n]  = sum_c wwT[c,o]·y[c,n]                (BN scale in ww)
    out    [o,n]  = w_y + (xh[o,n] + bnt[o])             (bnt folded into the
                                                          bf16 residual copy)

ALL six matmuls run as fp8e4 DoubleRow pairs at 0.5 PE-cycles per moving
row.  Each operand is split hi+lo in fp8 (double-fp8 ~11 mantissa bits) and
each product keeps 3 of the 4 cross terms:

    W^T X ~= W_hi^T(X_hi + X_lo) + W_lo^T X_hi            (lo·lo dropped)

One DoubleRow instr computes A^T i0 + B^T i1, so per 128-deep k-chunk the
paired-plane side does one instr per k and the lo-correction parts of two
adjacent k-chunks share one instr via strided APs.  Net 0.75 cycles per
128-contraction row vs 1.0 for fp32r, at ~0.4% rel error (gate is 2e-2).
Everything is pre-scaled by powers of two (weights x16, x x4, th x8,
ph x4096, g x16, att x64, y x2) so fp8 lo planes stay in e4m3 normal range
and every eviction absorbs the inverse scale in its scalar multiplier.
Device-side hi/lo splits (th/ph/g/att/y) cost one extra Act cast + one Pool
subtract per tile, spread so no engine paces the PE.  x_h arrives as
host-split fp8 planes plus a bf16 residual copy with the BN shift
pre-added; output is written bf16 and upcast on host.

Schedule notes: the PE p-state ramp is burned on dummy matmuls during the
prologue DMA wait; batch-0 theta inputs (thw, xh hi plane, then xh lo
plane) stream before phase-C inputs (ww, xhb), and theta's lo-correction
instrs are issued first since they only need the hi plane.
"""

import numpy as np

import concourse.bass as bass
import concourse.bacc as bacc
import concourse.mybir as mybir
import concourse.tile as tile
from concourse.bass import ts

B, HIGH, LOW, H, W = 16, 2048, 512, 48, 24
N = H * W            # 1152
NCORES = 8
BPC = B // NCORES    # 2 batches per core
P = 128
KH = HIGH // P       # 16
KL = LOW // P        # 4
MN = N // P          # 9
NSPLIT = 3
NW = N // NSPLIT     # 384
BN_EPS = 1e-5

SW = 16.0            # weight fp8 pre-scale
SX = 4.0             # x_h / x_l fp8 pre-scale
STH = 8.0            # theta_xT fp8 split scale
SPH = 4096.0         # phi_xT fp8 split scale (on top of the /512 fold)
SG = 16.0            # g_x fp8 split scale
SATT = 64.0          # att fp8 split scale
SY = 2.0             # y fp8 split scale

F32 = mybir.dt.float32
BF16 = mybir.dt.bfloat16
FP8 = mybir.dt.float8e4
DR = mybir.MatmulPerfMode.DoubleRow
ADD = mybir.AluOpType.add
MULT = mybir.AluOpType.mult
SUB = mybir.AluOpType.subtract
AF = mybir.ActivationFunctionType


def _dup(ap, n):
    # (W, W) stride-0 pair for one slot of a DoubleRow operand
    return ap.unsqueeze(1).to_broadcast((P, 2, n))


def _build_module() -> bass.Bass:
    nc = bacc.Bacc()
    xhf8 = nc.dram_tensor("xhf8", [BPC, P, 2, KH, N], FP8, kind="ExternalInput")
    xhb = nc.dram_tensor("xhb", [BPC, P, KH, N], BF16, kind="ExternalInput")
    xlf8 = nc.dram_tensor("xlf8", [BPC, P, 2, KL, N], FP8, kind="ExternalInput")
    thwh = nc.dram_tensor("thwh", [P, KH, LOW], FP8, kind="ExternalInput")
    thwl = nc.dram_tensor("thwl", [P, KH, LOW], FP8, kind="ExternalInput")
    phwh = nc.dram_tensor("phwh", [P, KL, LOW], FP8, kind="ExternalInput")
    phwl = nc.dram_tensor("phwl", [P, KL, LOW], FP8, kind="ExternalInput")
    gwh = nc.dram_tensor("gwh", [P, KL, LOW], FP8, kind="ExternalInput")
    gwl = nc.dram_tensor("gwl", [P, KL, LOW], FP8, kind="ExternalInput")
    wwh = nc.dram_tensor("wwh", [P, KL, HIGH], FP8, kind="ExternalInput")
    wwl = nc.dram_tensor("wwl", [P, KL, HIGH], FP8, kind="ExternalInput")
    thpb = nc.dram_tensor("thpb", [1, 2 * LOW], BF16, kind="ExternalInput")
    gb = nc.dram_tensor("gb", [P, KL], F32, kind="ExternalInput")
    bnt = nc.dram_tensor("bnt", [P, KH], F32, kind="ExternalInput")
    ident = nc.dram_tensor("ident", [P, 2, P], FP8, kind="ExternalInput")
    out = nc.dram_tensor("out", [BPC, HIGH, N], BF16, kind="ExternalOutput")

    s_a1 = STH / (SW * SX)             # A1 evict: psum -> 8*theta_xT   (1/8)
    s_a2 = SPH / (SW * SX * LOW)       # A2 evict: psum -> 4096*ph      (1/8)
    s_a3 = SG / (SW * SX)              # A3 evict: psum -> 16*g_x       (1/4)
    s_b1 = SATT / (STH * SPH)          # B1 evict: psum -> 64*att       (2^-9)
    s_b2 = SY / (SATT * SG)            # B2 evict: psum -> 2*y          (2^-9)
    inv_c = 1.0 / (SW * SY)            # C evict                        (1/32)

    with tile.TileContext(nc) as tc:
        with (
            tc.tile_pool(name="consts", bufs=1) as cpool,
            tc.tile_pool(name="xh8", bufs=2) as xh8pool,
            tc.tile_pool(name="xhb", bufs=4) as xhbpool,
            tc.tile_pool(name="xl", bufs=1) as xlpool,
            tc.tile_pool(name="mid", bufs=1) as midpool,
            tc.tile_pool(name="tmp", bufs=3) as tmppool,
            tc.tile_pool(name="stg", bufs=4) as stgpool,
            tc.tile_pool(name="psum", bufs=8, space="PSUM") as pspool,
        ):
            # prologue: biases first (the A2 evictions need them right after
            # the first matmuls), then phw, then xl in two n-halves: A2's
            # m-groups read only their own n-columns, so the first half
            # unblocks m=0..3
            thpb_sb = cpool.tile([P, 2 * LOW], BF16, tag="thpb")
            nc.sync.dma_start(thpb_sb[:], thpb[:].to_broadcast((P, 2 * LOW)))
            thb_sb = thpb_sb[:, :LOW]
            phb_sb = thpb_sb[:, LOW:]
            gb_sb = cpool.tile([P, KL], F32, tag="gb")
            nc.sync.dma_start(gb_sb[:], gb[:])
            phwh_sb = cpool.tile([P, KL, LOW], FP8, tag="phwh")
            nc.sync.dma_start(phwh_sb[:], phwh[:])
            phwl_sb = cpool.tile([P, KL, LOW], FP8, tag="phwl")
            nc.sync.dma_start(phwl_sb[:], phwl[:])
            xl0_sb = xlpool.tile([P, 2, KL, N], FP8, tag="xl")
            for h in range(2):
                nc.sync.dma_start(
                    xl0_sb[:, :, :, ts(h, N // 2)], xlf8[0, :, :, :, ts(h, N // 2)]
                )
            gwh_sb = cpool.tile([P, KL, LOW], FP8, tag="gwh")
            nc.sync.dma_start(gwh_sb[:], gwh[:])
            gwl_sb = cpool.tile([P, KL, LOW], FP8, tag="gwl")
            nc.sync.dma_start(gwl_sb[:], gwl[:])
            bnt_sb = cpool.tile([P, KH], F32, tag="bnt")
            nc.sync.dma_start(bnt_sb[:], bnt[:])
            ident_sb = cpool.tile([P, 2, P], FP8, tag="ident")
            nc.sync.dma_start(ident_sb[:], ident[:])
            # theta weights interleaved with batch-0 xh fp8 planes: these gate
            # phase A1 of batch 0, so they go before ww/xhb (phase-C inputs).
            # hi plane streams fully before lo: A1's lo-correction instrs only
            # need the hi plane, so issuing them first (below) lets the PE
            # keep pace with DMA arrival
            thwh_sb = cpool.tile([P, KH, LOW], FP8, tag="thwh")
            thwl_sb = cpool.tile([P, KH, LOW], FP8, tag="thwl")
            xh8_b0 = xh8pool.tile([P, 2, KH, N], FP8, tag="xh8")
            for q in range(4):
                nc.sync.dma_start(
                    thwh_sb[:, ts(q, KH // 4)], thwh[:, ts(q, KH // 4)]
                )
                nc.sync.dma_start(
                    thwl_sb[:, ts(q, KH // 4)], thwl[:, ts(q, KH // 4)]
                )
                nc.sync.dma_start(
                    xh8_b0[:, 0, ts(q, KH // 4)], xhf8[0, :, 0, ts(q, KH // 4)]
                )
            for q in range(4):
                nc.sync.dma_start(
                    xh8_b0[:, 1, ts(q, KH // 4)], xhf8[0, :, 1, ts(q, KH // 4)]
                )
            wwh_sb = cpool.tile([P, KL, HIGH], FP8, tag="wwh")
            nc.sync.dma_start(wwh_sb[:], wwh[:])
            wwl_sb = cpool.tile([P, KL, HIGH], FP8, tag="wwl")
            nc.sync.dma_start(wwl_sb[:], wwl[:])

            # warm the PE p-state during the prologue DMA wait: ~4.7us of dummy
            # matmuls on a memset tile burn the half-clock ramp window so real
            # work starts at full clock (sized to end just before xl lands)
            warm = cpool.tile([P, 640], FP8, tag="warm")
            nc.vector.memset(warm[:], 0.0)
            wps = pspool.tile([P, 512], F32, tag="ps")
            warm_l = warm[:, :256].rearrange("p (two m) -> p two m", two=2)
            warm_r = warm[:, :512].unsqueeze(1).to_broadcast((P, 2, 512))
            for i in range(34):
                nc.tensor.matmul(
                    wps[:],
                    warm_l,
                    warm_r,
                    start=(i == 0),
                    stop=(i == 33),
                    perf_mode=DR,
                )

            for b in range(BPC):
                if b == 0:
                    xl_sb = xl0_sb
                    xh8_sb = xh8_b0
                else:
                    xl_sb = xlpool.tile([P, 2, KL, N], FP8, tag="xl")
                    nc.sync.dma_start(xl_sb[:], xlf8[b])
                    xh8_sb = xh8pool.tile([P, 2, KH, N], FP8, tag="xh8")
                    for pl in range(2):
                        for q in range(4):
                            nc.sync.dma_start(
                                xh8_sb[:, pl, ts(q, KH // 4)],
                                xhf8[b, :, pl, ts(q, KH // 4)],
                            )
                xhb_t = []
                for q in range(4):
                    t_ = xhbpool.tile([P, KH // 4, N], BF16, tag="xhb")
                    nc.sync.dma_start(t_[:], xhb[b, :, ts(q, KH // 4)])
                    xhb_t.append(t_)

                # phi_xT planes [n, d] (A2): stationary xl pair, moving phw.
                # ph8 has a zeroed 10th k-slot so B1's odd lo-instr can pair
                # (ph_hi[8], 0)
                ph8 = midpool.tile([P, 2, MN + 1, LOW], FP8, tag="ph")
                nc.vector.memset(ph8[:, 0, MN, :], 0.0)
                for m in range(MN):
                    ps = pspool.tile([P, 512], F32, tag="ps")
                    for k in range(KL):
                        nc.tensor.matmul(
                            ps[:],
                            xl_sb[:, :, k, ts(m, P)],
                            _dup(phwh_sb[:, k, :], LOW),
                            start=(k == 0),
                            stop=False,
                            perf_mode=DR,
                        )
                    for j in range(KL // 2):
                        nc.tensor.matmul(
                            ps[:],
                            xl_sb[:, 0, ts(j, 2), ts(m, P)],
                            phwl_sb[:, ts(j, 2)],
                            start=False,
                            stop=(j == KL // 2 - 1),
                            perf_mode=DR,
                        )
                    tmp = tmppool.tile([P, 512], F32, tag="tmp")
                    nc.vector.scalar_tensor_tensor(
                        tmp[:], ps[:], s_a2, phb_sb, MULT, ADD
                    )
                    nc.scalar.activation(ph8[:, 0, m, :], tmp[:], AF.Copy)
                    nc.gpsimd.scalar_tensor_tensor(
                        ph8[:, 1, m, :], tmp[:], 1.0, ph8[:, 0, m, :], MULT, SUB
                    )

                # g_x planes [d, n] (A3): stationary gw pair, moving xl planes
                g8 = midpool.tile([P, 2, KL, N], FP8, tag="g")
                for md in range(KL):
                    for nn in range(NSPLIT):
                        ps = pspool.tile([P, 512], F32, tag="ps")
                        for k in range(KL):
                            nc.tensor.matmul(
                                ps[:, :NW],
                                _dup(gwh_sb[:, k, ts(md, P)], P),
                                xl_sb[:, :, k, ts(nn, NW)],
                                start=(k == 0),
                                stop=False,
                                perf_mode=DR,
                            )
                        for j in range(KL // 2):
                            nc.tensor.matmul(
                                ps[:, :NW],
                                gwl_sb[:, ts(j, 2), ts(md, P)],
                                xl_sb[:, 0, ts(j, 2), ts(nn, NW)],
                                start=False,
                                stop=(j == KL // 2 - 1),
                                perf_mode=DR,
                            )
                        # tmp on DVE (broadcast bias) — two Act ops per tile
                        # would pace A3 at 1010ns/tile vs the PE's 480ns
                        tmp = tmppool.tile([P, 512], F32, tag="tmp")
                        nc.vector.scalar_tensor_tensor(
                            tmp[:, :NW],
                            ps[:, :NW],
                            s_a3,
                            gb_sb[:, md : md + 1].to_broadcast((P, NW)),
                            MULT,
                            ADD,
                        )
                        nc.scalar.activation(
                            g8[:, 0, md, ts(nn, NW)], tmp[:, :NW], AF.Copy
                        )
                        nc.gpsimd.scalar_tensor_tensor(
                            g8[:, 1, md, ts(nn, NW)],
                            tmp[:, :NW],
                            1.0,
                            g8[:, 0, md, ts(nn, NW)],
                            MULT,
                            SUB,
                        )

                # theta_xT planes [n, c] (A1): stationary xh planes, moving
                # thw, as three pair-across-k instr sets (hi·Whi, hi·Wlo,
                # lo·Whi).  The first two read only the hi plane, which lands
                # before the lo plane in the DMA stream, so the lo-dependent
                # set issues last and the PE keeps pace with DMA arrival
                th8 = midpool.tile([P, 2, MN, LOW], FP8, tag="th")
                for m in range(MN):
                    ps = pspool.tile([P, 512], F32, tag="ps")
                    for j in range(KH // 2):
                        nc.tensor.matmul(
                            ps[:],
                            xh8_sb[:, 0, ts(j, 2), ts(m, P)],
                            thwh_sb[:, ts(j, 2)],
                            start=(j == 0),
                            stop=False,
                            perf_mode=DR,
                        )
                    for j in range(KH // 2):
                        nc.tensor.matmul(
                            ps[:],
                            xh8_sb[:, 0, ts(j, 2), ts(m, P)],
                            thwl_sb[:, ts(j, 2)],
                            start=False,
                            stop=False,
                            perf_mode=DR,
                        )
                    for j in range(KH // 2):
                        nc.tensor.matmul(
                            ps[:],
                            xh8_sb[:, 1, ts(j, 2), ts(m, P)],
                            thwh_sb[:, ts(j, 2)],
                            start=False,
                            stop=(j == KH // 2 - 1),
                            perf_mode=DR,
                        )
                    tmp = tmppool.tile([P, 512], F32, tag="tmp")
                    nc.vector.scalar_tensor_tensor(
                        tmp[:], ps[:], s_a1, thb_sb, MULT, ADD
                    )
                    nc.scalar.activation(th8[:, 0, m, :], tmp[:], AF.Copy)
                    nc.gpsimd.scalar_tensor_tensor(
                        th8[:, 1, m, :], tmp[:], 1.0, th8[:, 0, m, :], MULT, SUB
                    )

                # attT planes [d, c] = energy^T/512 (B1): stationary ph pair,
                # moving th planes; att parks in the xl slot
                att8 = xlpool.tile([P, 2, KL, LOW], FP8, tag="xl")
                for md in range(KL):
                    ps = pspool.tile([P, 512], F32, tag="ps")
                    for k in range(MN):
                        nc.tensor.matmul(
                            ps[:],
                            ph8[:, :, k, ts(md, P)],
                            _dup(th8[:, 0, k, :], LOW),
                            start=(k == 0),
                            stop=False,
                            perf_mode=DR,
                        )
                    for j in range(MN // 2):
                        nc.tensor.matmul(
                            ps[:],
                            ph8[:, 0, ts(j, 2), ts(md, P)],
                            th8[:, 1, ts(j, 2), :],
                            start=False,
                            stop=False,
                            perf_mode=DR,
                        )
                    # odd 9th chunk: lhsT pairs (ph_hi[8], zero-slot-9)
                    nc.tensor.matmul(
                        ps[:],
                        ph8[:, 0, MN - 1 : MN + 1, ts(md, P)],
                        _dup(th8[:, 1, MN - 1, :], LOW),
                        start=False,
                        stop=True,
                        perf_mode=DR,
                    )
                    nc.scalar.activation(
                        att8[:, 0, md, :], ps[:], AF.Copy, scale=s_b1
                    )
                    nc.vector.scalar_tensor_tensor(
                        att8[:, 1, md, :], ps[:], s_b1, att8[:, 0, md, :], MULT, SUB
                    )

                # y planes [c, n] (B2): stationary att pair, moving g planes;
                # parks in the th slot
                y8 = midpool.tile([P, 2, KL, N], FP8, tag="th")
                # nn-major: phase C consumes y8 low columns first, so finish
                # their hi/lo splits across all mc before moving right
                for nn in range(NSPLIT):
                    for mc in range(KL):
                        ps = pspool.tile([P, 512], F32, tag="ps")
                        for k in range(KL):
                            nc.tensor.matmul(
                                ps[:, :NW],
                                _dup(att8[:, 0, k, ts(mc, P)], P),
                                g8[:, :, k, ts(nn, NW)],
                                start=(k == 0),
                                stop=False,
                                perf_mode=DR,
                            )
                        for j in range(KL // 2):
                            nc.tensor.matmul(
                                ps[:, :NW],
                                att8[:, 1, ts(j, 2), ts(mc, P)],
                                g8[:, 0, ts(j, 2), ts(nn, NW)],
                                start=False,
                                stop=(j == KL // 2 - 1),
                                perf_mode=DR,
                            )
                        nc.scalar.activation(
                            y8[:, 0, mc, ts(nn, NW)], ps[:, :NW], AF.Copy, scale=s_b2
                        )
                        nc.vector.scalar_tensor_tensor(
                            y8[:, 1, mc, ts(nn, NW)],
                            ps[:, :NW],
                            s_b2,
                            y8[:, 0, mc, ts(nn, NW)],
                            MULT,
                            SUB,
                        )

                # w_y + residual  (C): stationary ww pair, moving y planes.
                # stage a full [P, N] row per mo -> one out DMA per mo (the
                # SP sequencer spends ~0.5us per dma_start; 48 issues/batch
                # would pace the whole phase)
                # segments [512, 512, 128]: the two wide pieces evict on
                # DVE with the bf16 residual; the 128 sliver rides the PE
                # (one DoubleRow instr with an 8I pair adds 32*xh into the
                # psum — GPSIMD cannot read PSUM) and Act evicts with the BN
                # shift as its bias.  The narrow Act segment minimizes the
                # ident instr cost (width/2 cycles) and the final drain
                CSEG = [(0, 512, "dve"), (512, 512, "dve"), (1024, 128, "act")]
                for mo in range(KH):
                    xt = xhb_t[mo // 4]
                    stg = stgpool.tile([P, N], BF16, tag="stg")
                    last = b == BPC - 1 and mo == KH - 1
                    for o0, w, path in CSEG:
                        ps = pspool.tile([P, 512], F32, tag="ps")
                        for k in range(KL):
                            nc.tensor.matmul(
                                ps[:, :w],
                                _dup(wwh_sb[:, k, ts(mo, P)], P),
                                y8[:, :, k, o0 : o0 + w],
                                start=(k == 0),
                                stop=False,
                                perf_mode=DR,
                            )
                        for j in range(KL // 2):
                            nc.tensor.matmul(
                                ps[:, :w],
                                wwl_sb[:, ts(j, 2), ts(mo, P)],
                                y8[:, 0, ts(j, 2), o0 : o0 + w],
                                start=False,
                                stop=(path == "dve" and j == KL // 2 - 1),
                                perf_mode=DR,
                            )
                        if path == "dve":
                            nc.vector.scalar_tensor_tensor(
                                stg[:, o0 : o0 + w],
                                ps[:, :w],
                                inv_c,
                                xt[:, mo % 4, o0 : o0 + w],
                                MULT,
                                ADD,
                            )
                        else:
                            nc.tensor.matmul(
                                ps[:, :w],
                                ident_sb[:],
                                xh8_sb[:, :, mo, o0 : o0 + w],
                                start=False,
                                stop=True,
                                perf_mode=DR,
                            )
                            nc.scalar.activation(
                                stg[:, o0 : o0 + w],
                                ps[:, :w],
                                AF.Identity,
                                bias=bnt_sb[:, mo : mo + 1],
                                scale=inv_c,
                            )
                        if last:
                            # per-segment DMAs: earlier pieces fly while the
                            # final 128-wide sliver drains
                            nc.sync.dma_start(
                                out[b, ts(mo, P), o0 : o0 + w], stg[:, o0 : o0 + w]
                            )
                    if not last:
                        nc.sync.dma_start(out[b, ts(mo, P), :], stg[:])
    nc.compile()
    return nc


_CACHE: dict = {}


def _get_module() -> bass.Bass:
    if "nc" not in _CACHE:
        _CACHE["nc"] = _build_module()
    return _CACHE["nc"]


def _split_fp8(x: np.ndarray, scale: float):
    import ml_dtypes

    E4 = ml_dtypes.float8_e4m3
    xs = (x * np.float32(scale)).astype(np.float32)
    hi = xs.astype(E4)
    lo = (xs - hi.astype(np.float32)).astype(E4)
    return hi, lo


def _prep_maps(inputs: dict) -> list[dict]:
    import ml_dtypes

    f = lambda a: np.ascontiguousarray(np.asarray(a, dtype=np.float32))
    x_h = f(inputs["x_h"]).reshape(B, HIGH, N)
    x_l = f(inputs["x_l"]).reshape(B, LOW, N)
    theta_w = f(inputs["theta_w"])
    phi_w = f(inputs["phi_w"])
    g_w = f(inputs["g_w"])
    w_w = f(inputs["w_w"])

    def wq(wmat, kk):
        # wmat [kk*P, F]; -> hi [P, kk, F] and lo [P, kk, F]
        hi, lo = _split_fp8(wmat, SW)
        hi = np.ascontiguousarray(hi.reshape(kk, P, -1).transpose(1, 0, 2))
        lo = np.ascontiguousarray(lo.reshape(kk, P, -1).transpose(1, 0, 2))
        return hi, lo

    thwh_h, thwl_h = wq(theta_w.T, KH)          # [HIGH, LOW] over h-chunks
    phwh_h, phwl_h = wq(phi_w.T, KL)            # [LOW, LOW]
    gwh_h, gwl_h = wq(g_w.T, KL)                # [LOW, LOW]
    s = f(inputs["bn_gamma"]) / np.sqrt(f(inputs["bn_var"]) + np.float32(BN_EPS))
    wwh_h, wwl_h = wq((w_w * s[:, None]).T, KL)  # [LOW, HIGH] over c-chunks

    def xq(x, kk):
        # x [Bn, kk*P, N] -> fp8 planes [Bn, P, 2, kk, N]
        hi, lo = _split_fp8(x, SX)
        hi = hi.reshape(-1, kk, P, N).transpose(0, 2, 1, 3)
        lo = lo.reshape(-1, kk, P, N).transpose(0, 2, 1, 3)
        return np.ascontiguousarray(np.stack([hi, lo], axis=2))

    xhf8_h = xq(x_h, KH)
    xlf8_h = xq(x_l, KL)

    t = (f(inputs["w_b"]) - f(inputs["bn_mean"])) * s + f(inputs["bn_beta"])
    xhb_h = np.ascontiguousarray(
        (x_h + t[None, :, None]).reshape(B, KH, P, N).transpose(0, 2, 1, 3)
    ).astype(ml_dtypes.bfloat16)
    bnt_h = np.ascontiguousarray(t.reshape(KH, P).T)
    ident_h = np.ascontiguousarray(
        np.broadcast_to((8.0 * np.eye(P, dtype=np.float32))[:, None, :], (P, 2, P))
    ).astype(ml_dtypes.float8_e4m3)

    # biases pre-scaled to the split scales: thb*STH, (phb/512)*SPH = 8*phb
    thpb_h = (
        np.concatenate(
            [
                f(inputs["theta_b"]) * np.float32(STH),
                f(inputs["phi_b"]) * np.float32(SPH / LOW),
            ]
        )
        .reshape(1, 2 * LOW)
        .astype(ml_dtypes.bfloat16)
    )
    gb_h = np.ascontiguousarray(
        (f(inputs["g_b"]) * np.float32(SG)).reshape(KL, P).T
    )

    shared = dict(
        thwh=thwh_h, thwl=thwl_h, phwh=phwh_h, phwl=phwl_h,
        gwh=gwh_h, gwl=gwl_h, wwh=wwh_h, wwl=wwl_h,
        thpb=thpb_h, gb=gb_h, bnt=bnt_h, ident=ident_h,
    )
    maps = []
    for c in range(NCORES):
        m = dict(shared)
        m["xhf8"] = np.ascontiguousarray(xhf8_h[c * BPC : (c + 1) * BPC])
        m["xlf8"] = np.ascontiguousarray(xlf8_h[c * BPC : (c + 1) * BPC])
        m["xhb"] = np.ascontiguousarray(xhb_h[c * BPC : (c + 1) * BPC])
        maps.append(m)
    return maps


def _run(inputs: dict, **kwargs):
    from concourse.bass_utils import run_bass_kernel_spmd

    nc = _get_module()
    in_maps = _prep_maps(inputs)
    res = run_bass_kernel_spmd(nc, in_maps, core_ids=list(range(NCORES)), **kwargs)
    parts = [np.asarray(r["out"], dtype=np.float32) for r in res.results]
    full = np.concatenate(parts, axis=0).reshape(B, HIGH, H, W)
    return full, res


def kernel(**inputs) -> np.ndarray:
    full, _ = _run(inputs)
    return full


# revision 43
# speedup vs baseline: 1.3959x; 1.0028x over previous
"""Trainium2 Bass kernel for the non-local-block module (nn_CNL_747324309589).

Sharding: data-parallel over batch — 16 batches across 8 NeuronCores, 2 per
core, no collectives.  Per batch (dims: HIGH=2048, LOW=512, N=H*W=1152):

    theta_xT[n,c] = sum_h xh[h,n]·thwT[h,c] + thb[c]
    phi_xT [n,d]  = sum_l xl[l,n]·phwT[l,d] + phb[d]     (evict folds /512)
    g_x    [d,n]  = sum_l gwT[l,d]·xl[l,n]  + gb[d]
    attT   [d,c]  = sum_n phi_xT[n,d]·theta_xT[n,c]      (= energy^T/512)
    y      [c,n]  = sum_d attT[d,c]·g_x[d,n]
    w_y    [o,n]  = sum_c wwT[c,o]·y[c,n]                (BN scale in ww)
    out    [o,n]  = w_y + (xh[o,n] + bnt[o])             (bnt folded into the
                                                          bf16 residual copy)

ALL six matmuls run as fp8e4 DoubleRow pairs at 0.5 PE-cycles per moving
row.  Each operand is split hi+lo in fp8 (double-fp8 ~11 mantissa bits) and
each product keeps 3 of the 4 cross terms:

    W^T X ~= W_hi^T(X_hi + X_lo) + W_lo^T X_hi            (lo·lo dropped)

One DoubleRow instr computes A^T i0 + B^T i1, so per 128-deep k-chunk the
paired-plane side does one instr per k and the lo-correction parts of two
adjacent k-chunks share one instr via strided APs.  Net 0.75 cycles per
128-contraction row vs 1.0 for fp32r, at ~0.4% rel error (gate is 2e-2).
Everything is pre-scaled by powers of two (weights x16, x x4, th x8,
ph x4096, g x16, att x64, y x2) so fp8 lo planes stay in e4m3 normal range
and every eviction absorbs the inverse scale in its scalar multiplier.
Device-side hi/lo splits (th/ph/g/att/y) cost one extra Act cast + one Pool
subtract per tile, spread so no engine paces the PE.  x_h arrives as
host-split fp8 planes plus a bf16 residual copy with the BN shift
pre-added; output is written bf16 and upcast on host.

Schedule notes: the PE p-state ramp is burned on dummy matmuls during the
prologue DMA wait; batch-0 theta inputs (thw, xh hi plane, then xh lo
plane) stream before phase-C inputs (ww, xhb), and theta's lo-correction
instrs are issued first since they only need the hi plane.
"""

import numpy as np

import concourse.bass as bass
import concourse.bacc as bacc
import concourse.mybir as mybir
import concourse.tile as tile
from concourse.bass import ts

B, HIGH, LOW, H, W = 16, 2048, 512, 48, 24
N = H * W            # 1152
NCORES = 8
BPC = B // NCORES    # 2 batches per core
P = 128
KH = HIGH // P       # 16
KL = LOW // P        # 4
MN = N // P          # 9
NSPLIT = 3
NW = N // NSPLIT     # 384
BN_EPS = 1e-5

SW = 16.0            # weight fp8 pre-scale
SX = 4.0             # x_h / x_l fp8 pre-scale
STH = 8.0            # theta_xT fp8 split scale
SPH = 4096.0         # phi_xT fp8 split scale (on top of the /512 fold)
SG = 16.0            # g_x fp8 split scale
SATT = 64.0          # att fp8 split scale
SY = 2.0             # y fp8 split scale

F32 = mybir.dt.float32
BF16 = mybir.dt.bfloat16
FP8 = mybir.dt.float8e4
DR = mybir.MatmulPerfMode.DoubleRow
ADD = mybir.AluOpType.add
MULT = mybir.AluOpType.mult
SUB = mybir.AluOpType.subtract
AF = mybir.ActivationFunctionType


def _dup(ap, n):
    # (W, W) stride-0 pair for one slot of a DoubleRow operand
    return ap.unsqueeze(1).to_broadcast((P, 2, n))


def _build_module() -> bass.Bass:
    nc = bacc.Bacc()
    xhf8 = nc.dram_tensor("xhf8", [BPC, P, 2, KH, N], FP8, kind="ExternalInput")
    xhb = nc.dram_tensor("xhb", [BPC, P, KH, N], BF16, kind="ExternalInput")
    xlf8 = nc.dram_tensor("xlf8", [BPC, P, 2, KL, N], FP8, kind="ExternalInput")
    thwh = nc.dram_tensor("thwh", [P, KH, LOW], FP8, kind="ExternalInput")
    thwl = nc.dram_tensor("thwl", [P, KH, LOW], FP8, kind="ExternalInput")
    phwh = nc.dram_tensor("phwh", [P, KL, LOW], FP8, kind="ExternalInput")
    phwl = nc.dram_tensor("phwl", [P, KL, LOW], FP8, kind="ExternalInput")
    gwh = nc.dram_tensor("gwh", [P, KL, LOW], FP8, kind="ExternalInput")
    gwl = nc.dram_tensor("gwl", [P, KL, LOW], FP8, kind="ExternalInput")
    wwh = nc.dram_tensor("wwh", [P, KL, HIGH], FP8, kind="ExternalInput")
    wwl = nc.dram_tensor("wwl", [P, KL, HIGH], FP8, kind="ExternalInput")
    thpb = nc.dram_tensor("thpb", [1, 2 * LOW], BF16, kind="ExternalInput")
    gb = nc.dram_tensor("gb", [P, KL], F32, kind="ExternalInput")
    bnt = nc.dram_tensor("bnt", [P, KH], F32, kind="ExternalInput")
    ident = nc.dram_tensor("ident", [P, 2, P], FP8, kind="ExternalInput")
    out = nc.dram_tensor("out", [BPC, HIGH, N], BF16, kind="ExternalOutput")

    s_a1 = STH / (SW * SX)             # A1 evict: psum -> 8*theta_xT   (1/8)
    s_a2 = SPH / (SW * SX * LOW)       # A2 evict: psum -> 4096*ph      (1/8)
    s_a3 = SG / (SW * SX)              # A3 evict: psum -> 16*g_x       (1/4)
    s_b1 = SATT / (STH * SPH)          # B1 evict: psum -> 64*att       (2^-9)
    s_b2 = SY / (SATT * SG)            # B2 evict: psum -> 2*y          (2^-9)
    inv_c = 1.0 / (SW * SY)            # C evict                        (1/32)

    with tile.TileContext(nc) as tc:
        with (
            tc.tile_pool(name="consts", bufs=1) as cpool,
            tc.tile_pool(name="xh8", bufs=2) as xh8pool,
            tc.tile_pool(name="xhb", bufs=4) as xhbpool,
            tc.tile_pool(name="xl", bufs=1) as xlpool,
            tc.tile_pool(name="mid", bufs=1) as midpool,
            tc.tile_pool(name="tmp", bufs=3) as tmppool,
            tc.tile_pool(name="stg", bufs=4) as stgpool,
            tc.tile_pool(name="psum", bufs=8, space="PSUM") as pspool,
        ):
            # prologue: biases first (the A2 evictions need them right after
            # the first matmuls), then phw, then xl in two n-halves: A2's
            # m-groups read only their own n-columns, so the first half
            # unblocks m=0..3
            thpb_sb = cpool.tile([P, 2 * LOW], BF16, tag="thpb")
            nc.sync.dma_start(thpb_sb[:], thpb[:].to_broadcast((P, 2 * LOW)))
            thb_sb = thpb_sb[:, :LOW]
            phb_sb = thpb_sb[:, LOW:]
            gb_sb = cpool.tile([P, KL], F32, tag="gb")
            nc.sync.dma_start(gb_sb[:], gb[:])
            phwh_sb = cpool.tile([P, KL, LOW], FP8, tag="phwh")
            nc.sync.dma_start(phwh_sb[:], phwh[:])
            phwl_sb = cpool.tile([P, KL, LOW], FP8, tag="phwl")
            nc.sync.dma_start(phwl_sb[:], phwl[:])
            xl0_sb = xlpool.tile([P, 2, KL, N], FP8, tag="xl")
            for h in range(2):
                nc.sync.dma_start(
                    xl0_sb[:, :, :, ts(h, N // 2)], xlf8[0, :, :, :, ts(h, N // 2)]
                )
            gwh_sb = cpool.tile([P, KL, LOW], FP8, tag="gwh")
            nc.sync.dma_start(gwh_sb[:], gwh[:])
            gwl_sb = cpool.tile([P, KL, LOW], FP8, tag="gwl")
            nc.sync.dma_start(gwl_sb[:], gwl[:])
            bnt_sb = cpool.tile([P, KH], F32, tag="bnt")
            nc.sync.dma_start(bnt_sb[:], bnt[:])
            ident_sb = cpool.tile([P, 2, P], FP8, tag="ident")
            nc.sync.dma_start(ident_sb[:], ident[:])
            # theta weights interleaved with batch-0 xh fp8 planes: these gate
            # phase A1 of batch 0, so they go before ww/xhb (phase-C inputs).
            # hi plane streams fully before lo: A1's lo-correction instrs only
            # need the hi plane, so issuing them first (below) lets the PE
            # keep pace with DMA arrival
            thwh_sb = cpool.tile([P, KH, LOW], FP8, tag="thwh")
            thwl_sb = cpool.tile([P, KH, LOW], FP8, tag="thwl")
            xh8_b0 = xh8pool.tile([P, 2, KH, N], FP8, tag="xh8")
            for q in range(4):
                nc.sync.dma_start(
                    thwh_sb[:, ts(q, KH // 4)], thwh[:, ts(q, KH // 4)]
                )
                nc.sync.dma_start(
                    thwl_sb[:, ts(q, KH // 4)], thwl[:, ts(q, KH // 4)]
                )
                nc.sync.dma_start(
                    xh8_b0[:, 0, ts(q, KH // 4)], xhf8[0, :, 0, ts(q, KH // 4)]
                )
            for q in range(4):
                nc.sync.dma_start(
                    xh8_b0[:, 1, ts(q, KH // 4)], xhf8[0, :, 1, ts(q, KH // 4)]
                )
            wwh_sb = cpool.tile([P, KL, HIGH], FP8, tag="wwh")
            nc.sync.dma_start(wwh_sb[:], wwh[:])
            wwl_sb = cpool.tile([P, KL, HIGH], FP8, tag="wwl")
            nc.sync.dma_start(wwl_sb[:], wwl[:])

            # warm the PE p-state during the prologue DMA wait: ~4.7us of dummy
            # matmuls on a memset tile burn the half-clock ramp window so real
            # work starts at full clock (sized to end just before xl lands)
            warm = cpool.tile([P, 640], FP8, tag="warm")
            nc.vector.memset(warm[:], 0.0)
            wps = pspool.tile([P, 512], F32, tag="ps")
            warm_l = warm[:, :256].rearrange("p (two m) -> p two m", two=2)
            warm_r = warm[:, :512].unsqueeze(1).to_broadcast((P, 2, 512))
            for i in range(34):
                nc.tensor.matmul(
                    wps[:],
                    warm_l,
                    warm_r,
                    start=(i == 0),
                    stop=(i == 33),
                    perf_mode=DR,
                )

            for b in range(BPC):
                if b == 0:
                    xl_sb = xl0_sb
                    xh8_sb = xh8_b0
                else:
                    xl_sb = xlpool.tile([P, 2, KL, N], FP8, tag="xl")
                    nc.sync.dma_start(xl_sb[:], xlf8[b])
                    xh8_sb = xh8pool.tile([P, 2, KH, N], FP8, tag="xh8")
                    for pl in range(2):
                        for q in range(4):
                            nc.sync.dma_start(
                                xh8_sb[:, pl, ts(q, KH // 4)],
                                xhf8[b, :, pl, ts(q, KH // 4)],
                            )
                xhb_t = []
                for q in range(4):
                    t_ = xhbpool.tile([P, KH // 4, N], BF16, tag="xhb")
                    nc.sync.dma_start(t_[:], xhb[b, :, ts(q, KH // 4)])
                    xhb_t.append(t_)

                # phi_xT planes [n, d] (A2): stationary xl pair, moving phw.
                # ph8 has a zeroed 10th k-slot so B1's odd lo-instr can pair
                # (ph_hi[8], 0)
                ph8 = midpool.tile([P, 2, MN + 1, LOW], FP8, tag="ph")
                nc.vector.memset(ph8[:, 0, MN, :], 0.0)
                for m in range(MN):
                    ps = pspool.tile([P, 512], F32, tag="ps")
                    for k in range(KL):
                        nc.tensor.matmul(
                            ps[:],
                            xl_sb[:, :, k, ts(m, P)],
                            _dup(phwh_sb[:, k, :], LOW),
                            start=(k == 0),
                            stop=False,
                            perf_mode=DR,
                        )
                    for j in range(KL // 2):
                        nc.tensor.matmul(
                            ps[:],
                            xl_sb[:, 0, ts(j, 2), ts(m, P)],
                            phwl_sb[:, ts(j, 2)],
                            start=False,
                            stop=(j == KL // 2 - 1),
                            perf_mode=DR,
                        )
                    tmp = tmppool.tile([P, 512], F32, tag="tmp")
                    nc.vector.scalar_tensor_tensor(
                        tmp[:], ps[:], s_a2, phb_sb, MULT, ADD
                    )
                    nc.scalar.activation(ph8[:, 0, m, :], tmp[:], AF.Copy)
                    nc.gpsimd.scalar_tensor_tensor(
                        ph8[:, 1, m, :], tmp[:], 1.0, ph8[:, 0, m, :], MULT, SUB
                    )

                # g_x planes [d, n] (A3): stationary gw pair, moving xl planes
                g8 = midpool.tile([P, 2, KL, N], FP8, tag="g")
                for md in range(KL):
                    for nn in range(NSPLIT):
                        ps = pspool.tile([P, 512], F32, tag="ps")
                        for k in range(KL):
                            nc.tensor.matmul(
                                ps[:, :NW],
                                _dup(gwh_sb[:, k, ts(md, P)], P),
                                xl_sb[:, :, k, ts(nn, NW)],
                                start=(k == 0),
                                stop=False,
                                perf_mode=DR,
                            )
                        for j in range(KL // 2):
                            nc.tensor.matmul(
                                ps[:, :NW],
                                gwl_sb[:, ts(j, 2), ts(md, P)],
                                xl_sb[:, 0, ts(j, 2), ts(nn, NW)],
                                start=False,
                                stop=(j == KL // 2 - 1),
                                perf_mode=DR,
                            )
                        # tmp on DVE (broadcast bias) — two Act ops per tile
                        # would pace A3 at 1010ns/tile vs the PE's 480ns
                        tmp = tmppool.tile([P, 512], F32, tag="tmp")
                        nc.vector.scalar_tensor_tensor(
                            tmp[:, :NW],
                            ps[:, :NW],
                            s_a3,
                            gb_sb[:, md : md + 1].to_broadcast((P, NW)),
                            MULT,
                            ADD,
                        )
                        nc.scalar.activation(
                            g8[:, 0, md, ts(nn, NW)], tmp[:, :NW], AF.Copy
                        )
                        nc.gpsimd.scalar_tensor_tensor(
                            g8[:, 1, md, ts(nn, NW)],
                            tmp[:, :NW],
                            1.0,
                            g8[:, 0, md, ts(nn, NW)],
                            MULT,
                            SUB,
                        )

                # theta_xT planes [n, c] (A1): stationary xh planes, moving
                # thw, as three pair-across-k instr sets (hi·Whi, hi·Wlo,
                # lo·Whi).  The first two read only the hi plane, which lands
                # before the lo plane in the DMA stream, so the lo-dependent
                # set issues last and the PE keeps pace with DMA arrival
                th8 = midpool.tile([P, 2, MN, LOW], FP8, tag="th")
                for m in range(MN):
                    ps = pspool.tile([P, 512], F32, tag="ps")
                    for j in range(KH // 2):
                        nc.tensor.matmul(
                            ps[:],
                            xh8_sb[:, 0, ts(j, 2), ts(m, P)],
                            thwh_sb[:, ts(j, 2)],
                            start=(j == 0),
                            stop=False,
                            perf_mode=DR,
                        )
                    for j in range(KH // 2):
                        nc.tensor.matmul(
                            ps[:],
                            xh8_sb[:, 0, ts(j, 2), ts(m, P)],
                            thwl_sb[:, ts(j, 2)],
                            start=False,
                            stop=False,
                            perf_mode=DR,
                        )
                    for j in range(KH // 2):
                        nc.tensor.matmul(
                            ps[:],
                            xh8_sb[:, 1, ts(j, 2), ts(m, P)],
                            thwh_sb[:, ts(j, 2)],
                            start=False,
                            stop=(j == KH // 2 - 1),
                            perf_mode=DR,
                        )
                    tmp = tmppool.tile([P, 512], F32, tag="tmp")
                    nc.vector.scalar_tensor_tensor(
                        tmp[:], ps[:], s_a1, thb_sb, MULT, ADD
                    )
                    nc.scalar.activation(th8[:, 0, m, :], tmp[:], AF.Copy)
                    nc.gpsimd.scalar_tensor_tensor(
                        th8[:, 1, m, :], tmp[:], 1.0, th8[:, 0, m, :], MULT, SUB
                    )

                # attT planes [d, c] = energy^T/512 (B1): stationary ph pair,
                # moving th planes; att parks in the xl slot
                att8 = xlpool.tile([P, 2, KL, LOW], FP8, tag="xl")
                for md in range(KL):
                    ps = pspool.tile([P, 512], F32, tag="ps")
                    for k in range(MN):
                        nc.tensor.matmul(
                            ps[:],
                            ph8[:, :, k, ts(md, P)],
                            _dup(th8[:, 0, k, :], LOW),
                            start=(k == 0),
                            stop=False,
                            perf_mode=DR,
                        )
                    for j in range(MN // 2):
                        nc.tensor.matmul(
                            ps[:],
                            ph8[:, 0, ts(j, 2), ts(md, P)],
                            th8[:, 1, ts(j, 2), :],
                            start=False,
                            stop=False,
                            perf_mode=DR,
                        )
                    # odd 9th chunk: lhsT pairs (ph_hi[8], zero-slot-9)
                    nc.tensor.matmul(
                        ps[:],
                        ph8[:, 0, MN - 1 : MN + 1, ts(md, P)],
                        _dup(th8[:, 1, MN - 1, :], LOW),
                        start=False,
                        stop=True,
                        perf_mode=DR,
                    )
                    nc.scalar.activation(
                        att8[:, 0, md, :], ps[:], AF.Copy, scale=s_b1
                    )
                    nc.vector.scalar_tensor_tensor(
                        att8[:, 1, md, :], ps[:], s_b1, att8[:, 0, md, :], MULT, SUB
                    )

                # y planes [c, n] (B2): stationary att pair, moving g planes;
                # parks in the th slot
                y8 = midpool.tile([P, 2, KL, N], FP8, tag="th")
                # nn-major: phase C consumes y8 low columns first, so finish
                # their hi/lo splits across all mc before moving right
                for nn in range(NSPLIT):
                    for mc in range(KL):
                        ps = pspool.tile([P, 512], F32, tag="ps")
                        for k in range(KL):
                            nc.tensor.matmul(
                                ps[:, :NW],
                                _dup(att8[:, 0, k, ts(mc, P)], P),
                                g8[:, :, k, ts(nn, NW)],
                                start=(k == 0),
                                stop=False,
                                perf_mode=DR,
                            )
                        for j in range(KL // 2):
                            nc.tensor.matmul(
                                ps[:, :NW],
                                att8[:, 1, ts(j, 2), ts(mc, P)],
                                g8[:, 0, ts(j, 2), ts(nn, NW)],
                                start=False,
                                stop=(j == KL // 2 - 1),
                                perf_mode=DR,
                            )
                        nc.scalar.activation(
                            y8[:, 0, mc, ts(nn, NW)], ps[:, :NW], AF.Copy, scale=s_b2
                        )
                        nc.vector.scalar_tensor_tensor(
                            y8[:, 1, mc, ts(nn, NW)],
                            ps[:, :NW],
                            s_b2,
                            y8[:, 0, mc, ts(nn, NW)],
                            MULT,
                            SUB,
                        )

                # w_y + residual  (C): stationary ww pair, moving y planes.
                # stage a full [P, N] row per mo -> one out DMA per mo (the
                # SP sequencer spends ~0.5us per dma_start; 48 issues/batch
                # would pace the whole phase)
                # segments [512, 512, 128]: the two wide pieces evict on
                # DVE with the bf16 residual; the 128 sliver rides the PE
                # (one DoubleRow instr with an 8I pair adds 32*xh into the
                # psum — GPSIMD cannot read PSUM) and Act evicts with the BN
                # shift as its bias.  The narrow Act segment minimizes the
                # ident instr cost (width/2 cycles) and the final drain
                CSEG = [(0, 512, "dve"), (512, 512, "dve"), (1024, 128, "act")]
                for mo in range(KH):
                    xt = xhb_t[mo // 4]
                    stg = stgpool.tile([P, N], BF16, tag="stg")
                    last = b == BPC - 1 and mo == KH - 1
                    for o0, w, path in CSEG:
                        ps = pspool.tile([P, 512], F32, tag="ps")
                        for k in range(KL):
                            nc.tensor.matmul(
                                ps[:, :w],
                                _dup(wwh_sb[:, k, ts(mo, P)], P),
                                y8[:, :, k, o0 : o0 + w],
                                start=(k == 0),
                                stop=False,
                                perf_mode=DR,
                            )
                        for j in range(KL // 2):
                            nc.tensor.matmul(
                                ps[:, :w],
                                wwl_sb[:, ts(j, 2), ts(mo, P)],
                                y8[:, 0, ts(j, 2), o0 : o0 + w],
                                start=False,
                                stop=(path == "dve" and j == KL // 2 - 1),
                                perf_mode=DR,
                            )
                        if path == "dve":
                            nc.vector.scalar_tensor_tensor(
                                stg[:, o0 : o0 + w],
                                ps[:, :w],
                                inv_c,
                                xt[:, mo % 4, o0 : o0 + w],
                                MULT,
                                ADD,
                            )
                        else:
                            nc.tensor.matmul(
                                ps[:, :w],
                                ident_sb[:],
                                xh8_sb[:, :, mo, o0 : o0 + w],
                                start=False,
                                stop=True,
                                perf_mode=DR,
                            )
                            nc.scalar.activation(
                                stg[:, o0 : o0 + w],
                                ps[:, :w],
                                AF.Identity,
                                bias=bnt_sb[:, mo : mo + 1],
                                scale=inv_c,
                            )
                        if last:
                            # per-segment DMAs: earlier pieces fly while the
                            # final 128-wide sliver drains
                            nc.sync.dma_start(
                                out[b, ts(mo, P), o0 : o0 + w], stg[:, o0 : o0 + w]
                            )
                    if not last:
                        nc.sync.dma_start(out[b, ts(mo, P), :], stg[:])
    nc.compile()
    return nc


_CACHE: dict = {}


def _get_module() -> bass.Bass:
    if "nc" not in _CACHE:
        _CACHE["nc"] = _build_module()
    return _CACHE["nc"]


def _split_fp8(x: np.ndarray, scale: float):
    import ml_dtypes

    E4 = ml_dtypes.float8_e4m3
    xs = (x * np.float32(scale)).astype(np.float32)
    hi = xs.astype(E4)
    lo = (xs - hi.astype(np.float32)).astype(E4)
    return hi, lo


def _prep_maps(inputs: dict) -> list[dict]:
    import ml_dtypes

    f = lambda a: np.ascontiguousarray(np.asarray(a, dtype=np.float32))
    x_h = f(inputs["x_h"]).reshape(B, HIGH, N)
    x_l = f(inputs["x_l"]).reshape(B, LOW, N)
    theta_w = f(inputs["theta_w"])
    phi_w = f(inputs["phi_w"])
    g_w = f(inputs["g_w"])
    w_w = f(inputs["w_w"])

    def wq(wmat, kk):
        # wmat [kk*P, F]; -> hi [P, kk, F] and lo [P, kk, F]
        hi, lo = _split_fp8(wmat, SW)
        hi = np.ascontiguousarray(hi.reshape(kk, P, -1).transpose(1, 0, 2))
        lo = np.ascontiguousarray(lo.reshape(kk, P, -1).transpose(1, 0, 2))
        return hi, lo

    thwh_h, thwl_h = wq(theta_w.T, KH)          # [HIGH, LOW] over h-chunks
    phwh_h, phwl_h = wq(phi_w.T, KL)            # [LOW, LOW]
    gwh_h, gwl_h = wq(g_w.T, KL)                # [LOW, LOW]
    s = f(inputs["bn_gamma"]) / np.sqrt(f(inputs["bn_var"]) + np.float32(BN_EPS))
    wwh_h, wwl_h = wq((w_w * s[:, None]).T, KL)  # [LOW, HIGH] over c-chunks

    def xq(x, kk):
        # x [Bn, kk*P, N] -> fp8 planes [Bn, P, 2, kk, N]
        hi, lo = _split_fp8(x, SX)
        hi = hi.reshape(-1, kk, P, N).transpose(0, 2, 1, 3)
        lo = lo.reshape(-1, kk, P, N).transpose(0, 2, 1, 3)
        return np.ascontiguousarray(np.stack([hi, lo], axis=2))

    xhf8_h = xq(x_h, KH)
    xlf8_h = xq(x_l, KL)

    t = (f(inputs["w_b"]) - f(inputs["bn_mean"])) * s + f(inputs["bn_beta"])
    xhb_h = np.ascontiguousarray(
        (x_h + t[None, :, None]).reshape(B, KH, P, N).transpose(0, 2, 1, 3)
    ).astype(ml_dtypes.bfloat16)
    bnt_h = np.ascontiguousarray(t.reshape(KH, P).T)
    ident_h = np.ascontiguousarray(
        np.broadcast_to((8.0 * np.eye(P, dtype=np.float32))[:, None, :], (P, 2, P))
    ).astype(ml_dtypes.float8_e4m3)

    # biases pre-scaled to the split scales: thb*STH, (phb/512)*SPH = 8*phb
    thpb_h = (
        np.concatenate(
            [
                f(inputs["theta_b"]) * np.float32(STH),
                f(inputs["phi_b"]) * np.float32(SPH / LOW),
            ]
        )
        .reshape(1, 2 * LOW)
        .astype(ml_dtypes.bfloat16)
    )
    gb_h = np.ascontiguousarray(
        (f(inputs["g_b"]) * np.float32(SG)).reshape(KL, P).T
    )

    shared = dict(
        thwh=thwh_h, thwl=thwl_h, phwh=phwh_h, phwl=phwl_h,
        gwh=gwh_h, gwl=gwl_h, wwh=wwh_h, wwl=wwl_h,
        thpb=thpb_h, gb=gb_h, bnt=bnt_h, ident=ident_h,
    )
    maps = []
    for c in range(NCORES):
        m = dict(shared)
        m["xhf8"] = np.ascontiguousarray(xhf8_h[c * BPC : (c + 1) * BPC])
        m["xlf8"] = np.ascontiguousarray(xlf8_h[c * BPC : (c + 1) * BPC])
        m["xhb"] = np.ascontiguousarray(xhb_h[c * BPC : (c + 1) * BPC])
        maps.append(m)
    return maps


def _run(inputs: dict, **kwargs):
    from concourse.bass_utils import run_bass_kernel_spmd

    nc = _get_module()
    in_maps = _prep_maps(inputs)
    res = run_bass_kernel_spmd(nc, in_maps, core_ids=list(range(NCORES)), **kwargs)
    parts = [np.asarray(r["out"], dtype=np.float32) for r in res.results]
    full = np.concatenate(parts, axis=0).reshape(B, HIGH, H, W)
    return full, res


def kernel(**inputs) -> np.ndarray:
    full, _ = _run(inputs)
    return full
